# revision 25
# baseline (speedup 1.0000x reference)
"""Bass/Tile TRN2 kernel: pairwise-MLP multi-head attention (B=2,T=256,C=128,H=4,HS=32).

Sharding: 8 cores = (batch b in {0,1}) x (query residue k in {0..3}); core
(b, k) owns the 64 queries i == k (mod 4), so every core sees the same mix of
causal extents (32 queries with j<128, 32 with j<256) and the SPMD program is
identical across cores.

Per-core dataflow:
  pre[c,(ip,j)] = fp8 DoubleRow matmuls: (W1p_lo | W1k) and (W1p_hi | W1q)
                  each contract 256 rows in one PE instruction (0.5 cyc/col).
                  pos_dist arrives channel-major (pre-transposed + fp8 on CPU),
                  x1-key and x1-query broadcasts are prebuilt fp8 columns so the
                  kt-jump access pattern pairs them with the pdt halves.
  g = gelu(pre + b1[h])                        (ScalarE, psum -> sbuf bf16)
  score_t[j,i] = g_chunk.T @ w2[h]             (PE, g stationary, bf16)
  P_t = exp(scale*score_t + b2*scale) * mask   (ScalarE + DVE)
  out[i,:] = P_t.T @ [v | 1]; out /= Z         (PE; Z rides as v's 33rd column)

Causal skip: queries i<128 only compute/load the j<128 half -> 25% less PE,
Scalar and DMA work, identically on every core.
"""

import sys
from contextlib import ExitStack

import numpy as np

for _p in ("/opt/trn_rl_repo", "/root/.axon_site/_ro/trn_rl_repo"):
    if _p not in sys.path:
        sys.path.append(_p)

import ml_dtypes

import concourse.bass as bass
import concourse.mybir as mybir
import concourse.tile as tile
from concourse.bass_utils import run_bass_kernel_spmd

B, T, C = 2, 256, 128
H, HS = 4, 32
IBLK = 64            # queries per core
NCORES = 8
SCALE = float(C) ** -0.5

F32 = mybir.dt.float32
BF16 = mybir.dt.bfloat16
F8 = mybir.dt.float8e4
DR = mybir.MatmulPerfMode.DoubleRow

GELU = mybir.ActivationFunctionType.Gelu
EXP = mybir.ActivationFunctionType.Exp

# big-tile column layout (fp8, per partition).  The x1 key rows sit between
# the pdt halves so every kt-jump stride fits the 16-bit ISA step field.
PDT0 = 0                 # pdt kt0: 32 short q * 128 j, then 32 long q * 256 j
XTS = 12288              # x1 key row, j<128, duplicated for both pair queries
XTL = 12544              # x1 key row, full 256 j, duplicated
PDT1 = 13056             # pdt kt1: same layout as kt0
XQB = 25344              # x1 query-broadcast columns, same pair layout as pdt
NBIG = 37632
DB = XQB - PDT1          # constant kt-jump for the (W1p_hi | W1q) matmul

_build_cache = {}


def _legalize_single_wait(bir_json):
    """Split multi-wait instructions into single-wait NoOps + instruction.

    This walrus build's codegen (setupSyncWait) accepts at most one sem wait
    per ISA struct, but Tile's sem-assignment attaches wait *lists*.  Waits
    are ANDed and executed in order by the issuing sequencer, so hoisting all
    but one onto same-engine NoOps immediately before is semantically
    identical.
    """
    import json as _json

    m = _json.loads(bir_json)
    for fn in m.get("functions", []):
        for blk in fn.get("blocks", []):
            new = []
            for ins in blk.get("instructions", []):
                si = ins.get("sync_info")
                waits = (si or {}).get("on_wait") or []
                if len(waits) > 1:
                    for k, w in enumerate(waits[:-1]):
                        nop = {
                            "debug": ins.get("debug", 0),
                            "engine": ins["engine"],
                            "ins": [],
                            "name": f"{ins['name']}-ws{k}",
                            "opcode": "NoOp",
                            "outs": [],
                            "sync_info": {"on_wait": [w], "on_update": []},
                        }
                        new.append(nop)
                    si = dict(si)
                    si["on_wait"] = [waits[-1]]
                    ins = dict(ins)
                    ins["sync_info"] = si
                new.append(ins)
            blk["instructions"] = new
    return _json.dumps(m).encode()


def _install_wait_legalizer():
    from concourse import bass2jax as _b2j
    from concourse import bass_utils as _bu

    if getattr(_b2j, "_single_wait_patched", False):
        return
    _orig = _bu.compile_bir_kernel

    def _patched(bir_json, tmpdir, neff_name="file.neff"):
        return _orig(_legalize_single_wait(bir_json), tmpdir, neff_name)

    _b2j.compile_bir_kernel = _patched
    _b2j._single_wait_patched = True


def _bcast_ap(ap2d, count, pos):
    """Insert a [0, count] broadcast dim at free position `pos`."""
    dims = [list(d) for d in ap2d.ap]
    dims.insert(pos, [0, count])
    return bass.AP(tensor=ap2d.tensor, offset=ap2d.offset, ap=dims)


def _ktjump(tile_ap, off, delta, ncols):
    """rhs AP [128][kt: stride delta, 2][1, ncols] rooted at column `off`."""
    sl = tile_ap[:, off : off + 1]
    return bass.AP(
        tensor=sl.tensor,
        offset=sl.offset,
        ap=[list(sl.ap[0]), [delta, 2], [1, ncols]],
    )


def _build(b2_scaled):
    nc = bass.Bass()

    # DMA carries only pdt + x1 rows (XQB is pure replication, built on-chip)
    big = nc.dram_tensor("big", (128, XQB), F8, kind="ExternalInput")
    # wabq: wab [c, a/b, kt, h, m] flattened | xq;  cpack: w2 | wv | xt | mask
    wabq = nc.dram_tensor("wabq", (128, 2048 + IBLK), F8, kind="ExternalInput")
    cpack = nc.dram_tensor("cpack", (128, 516), BF16, kind="ExternalInput")
    b1t = nc.dram_tensor("b1t", (C, H), F32, kind="ExternalInput")
    out = nc.dram_tensor("out", (IBLK, H * HS), F32, kind="ExternalOutput")

    with tile.TileContext(nc) as tc, ExitStack() as ctx:
        const = ctx.enter_context(tc.tile_pool(name="const", bufs=1))
        gpool = ctx.enter_context(tc.tile_pool(name="gpool", bufs=4))
        psg = ctx.enter_context(tc.tile_pool(name="psg", bufs=2, space="PSUM"))
        pssc = ctx.enter_context(tc.tile_pool(name="pssc", bufs=1, space="PSUM"))
        psepi = ctx.enter_context(tc.tile_pool(name="psepi", bufs=1, space="PSUM"))

        # ---------- constants (DMAs spread over the 3 DMA-capable queues so
        # triggers (~0.6us each) and transfers overlap; each queue issues its
        # first-group-critical chunks first) ----------
        wq_sb = const.tile([128, 2048 + IBLK], F8)
        cp_sb = const.tile([128, 516], BF16)
        b1_sb = const.tile([C, H], F32)
        big_sb = const.tile([128, NBIG], F8)

        msl = cp_sb[:, 388:389]
        mask_bc = bass.AP(tensor=msl.tensor, offset=msl.offset,
                          ap=[list(msl.ap[0]), [64, 2], [0, H], [1, 64]])

        def wsl(ab, h):
            """lhsT view [c, kt, 128] for the a/b weight pack of head h."""
            sl = wq_sb[:, ab * 1024 + h * 128 : ab * 1024 + h * 128 + 1]
            return bass.AP(tensor=sl.tensor, offset=sl.offset,
                           ap=[list(sl.ap[0]), [512, 2], [1, 128]])

        # Banded DMA: each tensor is split into 3 partition-row bands, one per
        # trigger queue, so a "wave" costs ~43 packets instead of 128.
        BANDS = ((0, 43), (43, 86), (86, 128))
        QUEUES = (nc.gpsimd, nc.sync, nc.scalar)

        def band_all(dram, sbuf, a, b):
            for eng, (p0, p1) in zip(QUEUES, BANDS):
                eng.dma_start(out=sbuf[p0:p1, a:b], in_=dram[p0:p1, a:b])

        band_all(wabq, wq_sb, 0, 2048 + IBLK)
        band_all(big, big_sb, PDT0, PDT0 + 1024)
        band_all(big, big_sb, XTS, PDT1)
        band_all(big, big_sb, PDT1, PDT1 + 1024)
        band_all(b1t, b1_sb, 0, H)
        band_all(cpack, cp_sb, 0, 516)
        band_all(big, big_sb, PDT0 + 1024, PDT0 + 4096)
        band_all(big, big_sb, PDT1 + 1024, PDT1 + 4096)
        band_all(big, big_sb, PDT0 + 4096, XTS)
        band_all(big, big_sb, PDT1 + 4096, XQB)

        # x1 query-broadcast columns: replicate on-chip (DVE), in the order
        # the pair loop consumes them.
        for p in range(32):
            if p < 16:
                off, ext = XQB + p * 256, 128
            else:
                off, ext = XQB + 4096 + (p - 16) * 512, 256
            src = wq_sb[:, 2048 + 2 * p : 2048 + 2 * p + 1]
            src_bc = bass.AP(tensor=src.tensor, offset=src.offset,
                             ap=[list(src.ap[0]), [1, 2], [0, ext]])
            dst = big_sb[:, off : off + 1]
            dst_ap = bass.AP(tensor=dst.tensor, offset=dst.offset,
                             ap=[list(dst.ap[0]), [ext, 2], [1, ext]])
            nc.vector.tensor_copy(dst_ap, src_bc)

        pt_sb = const.tile([128, 2, H, IBLK], BF16)
        v_sb = const.tile([128, 2, H, HS + 1], BF16)
        recip = const.tile([IBLK, H, 1], F32)
        final_sb = const.tile([IBLK, H * HS], F32)

        # score accumulator [j%128, jb, h, i] - 1 PSUM bank, memset so the
        # never-written j-tiles of short queries exp() to a finite value.
        score_ps = pssc.tile([128, 2, H, IBLK], F32)
        nc.vector.memset(score_ps, 0.0)
        nc.vector.memset(v_sb[:, :, :, HS : HS + 1], 1.0)

        # ---------- v = x @ Wv (+ ones column for Z) ----------
        for jc in range(2):
            v_ps = psepi.tile([128, H, HS], F32, tag="vps", name=f"v{jc}")
            nc.tensor.matmul(v_ps, lhsT=cp_sb[:, 132 + jc * 128 : 260 + jc * 128],
                             rhs=cp_sb[:, 4:132], start=True, stop=True)
            nc.vector.tensor_copy(v_sb[:, jc, :, 0:HS], v_ps)

        # ---------- main loop ----------
        # groups: 4 short super-groups (8 queries, ext 128) + 8 long (4 q, 256)
        groups = [("s", sg) for sg in range(4)] + [("l", lg) for lg in range(8)]
        pending = []

        def emit_scores(g_t, kind, gi, h):
            # g_t rows: short = 8 queries x (jb=0); long = 4 queries x 2 jb
            nq = 8 if kind == "s" else 4
            njb = 1 if kind == "s" else 2
            q0 = 8 * gi if kind == "s" else 32 + 4 * gi
            for qi in range(nq):
                q = q0 + qi
                for jb in range(njb):
                    nc.tensor.matmul(
                        score_ps[:, jb, h, q : q + 1],
                        lhsT=g_t[:, qi * njb + jb, :],
                        rhs=cp_sb[:, h : h + 1],
                        start=True, stop=True,
                    )

        for kind, gi in groups:
            if kind == "s":
                npair, ext = 4, 128
                pairs = [4 * gi + pp for pp in range(4)]
            else:
                npair, ext = 2, 256
                pairs = [16 + 2 * gi + pp for pp in range(2)]
            for h in range(H):
                ps = psg.tile([128, 8, 128], F32, tag="pre", name=f"ps{kind}{gi}_{h}")
                for pp, p in enumerate(pairs):
                    if kind == "s":
                        o0 = PDT0 + p * 256
                        da = XTS - o0
                        out_sl = ps[:, 2 * pp : 2 * pp + 2, :]
                    else:
                        o0 = PDT0 + 4096 + (p - 16) * 512
                        da = XTL - o0
                        out_sl = ps[:, 4 * pp : 4 * pp + 4, :]
                    nc.tensor.matmul(out_sl, lhsT=wsl(0, h),
                                     rhs=_ktjump(big_sb, o0, da, 2 * ext),
                                     start=True, stop=False, perf_mode=DR)
                    nc.tensor.matmul(out_sl, lhsT=wsl(1, h),
                                     rhs=_ktjump(big_sb, o0 + PDT1, DB, 2 * ext),
                                     start=False, stop=True, perf_mode=DR)
                g_t = gpool.tile([128, 8, 128], BF16, tag="g", name=f"g{kind}{gi}_{h}")
                nc.scalar.activation(out=g_t, in_=ps, func=GELU,
                                     bias=b1_sb[:, h : h + 1], scale=1.0)
                pending.append((g_t, kind, gi, h))
                if len(pending) > 2:
                    emit_scores(*pending.pop(0))
        while pending:
            emit_scores(*pending.pop(0))

        # ---------- epilogue ----------
        for h in range(H):
            nc.scalar.activation(out=pt_sb[:, :, h, :], in_=score_ps[:, :, h, :],
                                 func=EXP, bias=float(b2_scaled[h]), scale=SCALE)
        nc.vector.tensor_mul(pt_sb, pt_sb, mask_bc)

        av = psepi.tile([IBLK, H, HS + 1], F32, tag="av", name="av")
        for h in range(H):
            for jc in range(2):
                nc.tensor.matmul(av[:, h, :], lhsT=pt_sb[:, jc, h, :],
                                 rhs=v_sb[:, jc, h, :],
                                 start=(jc == 0), stop=(jc == 1))
        nc.vector.reciprocal(out=recip, in_=av[:, :, HS : HS + 1])
        for h in range(H):
            nc.vector.tensor_scalar_mul(
                final_sb[:, h * HS : (h + 1) * HS], av[:, h, 0:HS], recip[:, h, :]
            )
        for eng, (p0, p1) in zip(QUEUES, ((0, 22), (22, 44), (44, IBLK))):
            eng.dma_start(out=out[p0:p1, :], in_=final_sb[p0:p1, :])

    return nc


def _prep_core(x1t_b, pd_b, k):
    """Build the per-core big-tile columns (fp8) and mask for residue k."""
    f8 = ml_dtypes.float8_e4m3fn
    qsel = 4 * np.arange(IBLK) + k
    arr = pd_b[qsel].transpose(2, 0, 1)            # (256 c2, 64 q, 256 j)
    regions = []
    for kt in range(2):
        ktarr = arr[kt * 128 : (kt + 1) * 128]
        short = ktarr[:, 0:32, 0:128].reshape(128, 4096)
        longr = ktarr[:, 32:64, :].reshape(128, 8192)
        regions.append(np.concatenate([short, longr], axis=1))
    x1qs = np.ascontiguousarray(x1t_b[:, qsel]).astype(f8)   # (128, 64)
    xts = np.tile(x1t_b[:, 0:128], (1, 2))
    xtl = np.tile(x1t_b, (1, 2))
    bigc = np.concatenate(
        [regions[0], xts, xtl, regions[1]], axis=1).astype(f8)

    jidx = np.arange(128)[:, None, None] + np.array([0, 128])[None, :, None]
    mask = (jidx <= (4 * np.arange(IBLK) + k)[None, None, :]).astype(
        ml_dtypes.bfloat16)
    return bigc, x1qs, mask


def kernel(**inputs):
    x = np.asarray(inputs["x"], np.float32)
    st = np.asarray(inputs["st_pos_emb"], np.float32)
    pd = np.asarray(inputs["pos_dist_emb"], np.float32)
    W1 = np.asarray(inputs["W1"], np.float32)
    b1 = np.asarray(inputs["b1"], np.float32)
    W2 = np.asarray(inputs["W2"], np.float32)
    b2 = np.asarray(inputs["b2"], np.float32)
    Wv = np.asarray(inputs["Wv"], np.float32)
    bv = np.asarray(inputs["bv"], np.float32)

    bf = ml_dtypes.bfloat16
    f8 = ml_dtypes.float8_e4m3fn
    x1 = x + st[None]                                    # (B, T, C)
    x1t_b = np.ascontiguousarray(x1.transpose(0, 2, 1))  # (B, C, T)

    W1k = W1[:, :C, :]                                   # (H, C, C)
    W1q = W1[:, C : 2 * C, :]
    W1p = W1[:, 2 * C :, :]                              # (H, 2C, C)
    wa_a = np.stack([W1p[:, 0:128, :], W1k], axis=0)     # (kt, H, c, m)
    wb_a = np.stack([W1p[:, 128:256, :], W1q], axis=0)
    wab_a = np.ascontiguousarray(
        np.stack([wa_a, wb_a], axis=0).transpose(3, 0, 1, 2, 4)
    ).astype(f8)                                         # (c, a/b, kt, h, m)
    w2_a = np.ascontiguousarray(W2.T).astype(bf)         # (C, H)
    b1_a = np.ascontiguousarray(b1.T)                    # (C, H)
    wv_a = Wv.transpose(1, 0, 2).reshape(C, H * HS).astype(bf)

    key = tuple(float(v) * SCALE for v in b2)
    if key not in _build_cache:
        _build_cache[key] = _build(key)
    nc = _build_cache[key]

    in_maps = []
    for core in range(NCORES):
        b, k = divmod(core, 4)
        bigc, x1qs, mask = _prep_core(x1t_b[b], pd[b], k)
        cpack = np.concatenate(
            [w2_a, wv_a, x[b].T.astype(bf), mask.reshape(128, 128)], axis=1)
        in_maps.append({
            "big": bigc,
            "wabq": np.ascontiguousarray(
                np.concatenate([wab_a.reshape(128, 2048), x1qs], axis=1)),
            "cpack": np.ascontiguousarray(cpack), "b1t": b1_a,
        })

    _install_wait_legalizer()
    res = run_bass_kernel_spmd(nc, in_maps, core_ids=list(range(NCORES)))
    outp = np.zeros((B, T, H * HS), np.float32)
    for core in range(NCORES):
        b, k = divmod(core, 4)
        outp[b, 4 * np.arange(IBLK) + k] = res.results[core]["out"]
    outp += bv.reshape(-1)[None, None, :]
    return outp


# revision 28
# speedup vs baseline: 1.5674x; 1.5674x over previous
"""Bass/Tile TRN2 kernel: pairwise-MLP multi-head attention (B=2,T=256,C=128,H=4,HS=32).

Sharding: 8 cores = (batch b in {0,1}) x (query residue k in {0..3}); core
(b, k) owns the 64 queries i == k (mod 4), so every core sees the same mix of
causal extents (32 queries with j<128, 32 with j<256) and the SPMD program is
identical across cores.

Per-core dataflow:
  pre[c,(ip,j)] = fp8 DoubleRow matmuls: (W1p_lo | W1k) and (W1p_hi | W1q)
                  each contract 256 rows in one PE instruction (0.5 cyc/col).
                  pos_dist arrives channel-major (pre-transposed + fp8 on CPU),
                  x1-key and x1-query broadcasts are prebuilt fp8 columns so the
                  kt-jump access pattern pairs them with the pdt halves.
  g = gelu(pre + b1[h])                        (ScalarE, psum -> sbuf bf16)
  score_t[j,i] = g_chunk.T @ w2[h]             (PE, g stationary, bf16)
  P_t = exp(scale*score_t + b2*scale) * mask   (ScalarE + DVE)
  out[i,:] = P_t.T @ [v | 1]; out /= Z         (PE; Z rides as v's 33rd column)

Causal skip: queries i<128 only compute/load the j<128 half -> 25% less PE,
Scalar and DMA work, identically on every core.
"""

import sys
from contextlib import ExitStack

import numpy as np

for _p in ("/opt/trn_rl_repo", "/root/.axon_site/_ro/trn_rl_repo"):
    if _p not in sys.path:
        sys.path.append(_p)

import ml_dtypes

import concourse.bass as bass
import concourse.mybir as mybir
import concourse.tile as tile
from concourse.bass_utils import run_bass_kernel_spmd

B, T, C = 2, 256, 128
H, HS = 4, 32
IBLK = 64            # queries per core
NCORES = 8
SCALE = float(C) ** -0.5

F32 = mybir.dt.float32
BF16 = mybir.dt.bfloat16
F8 = mybir.dt.float8e4
DR = mybir.MatmulPerfMode.DoubleRow

GELU = mybir.ActivationFunctionType.Gelu
EXP = mybir.ActivationFunctionType.Exp

# big-tile column layout (fp8, per partition).  The x1 key rows sit between
# the pdt halves so every kt-jump stride fits the 16-bit ISA step field.
PDT0 = 0                 # pdt kt0: 32 short q * 128 j, then 32 long q * 256 j
XTS = 12288              # x1 key row, j<128, duplicated for both pair queries
XTL = 12544              # x1 key row, full 256 j, duplicated
PDT1 = 13056             # pdt kt1: same layout as kt0
XQB = 25344              # x1 query-broadcast columns, same pair layout as pdt
NBIG = 37632
DB = XQB - PDT1          # constant kt-jump for the (W1p_hi | W1q) matmul

_build_cache = {}


def _legalize_single_wait(bir_json):
    """Split multi-wait instructions into single-wait NoOps + instruction.

    This walrus build's codegen (setupSyncWait) accepts at most one sem wait
    per ISA struct, but Tile's sem-assignment attaches wait *lists*.  Waits
    are ANDed and executed in order by the issuing sequencer, so hoisting all
    but one onto same-engine NoOps immediately before is semantically
    identical.
    """
    import json as _json

    m = _json.loads(bir_json)
    for fn in m.get("functions", []):
        for blk in fn.get("blocks", []):
            new = []
            for ins in blk.get("instructions", []):
                si = ins.get("sync_info")
                waits = (si or {}).get("on_wait") or []
                if len(waits) > 1:
                    for k, w in enumerate(waits[:-1]):
                        nop = {
                            "debug": ins.get("debug", 0),
                            "engine": ins["engine"],
                            "ins": [],
                            "name": f"{ins['name']}-ws{k}",
                            "opcode": "NoOp",
                            "outs": [],
                            "sync_info": {"on_wait": [w], "on_update": []},
                        }
                        new.append(nop)
                    si = dict(si)
                    si["on_wait"] = [waits[-1]]
                    ins = dict(ins)
                    ins["sync_info"] = si
                new.append(ins)
            blk["instructions"] = new
    return _json.dumps(m).encode()


def _install_wait_legalizer():
    from concourse import bass2jax as _b2j
    from concourse import bass_utils as _bu

    if getattr(_b2j, "_single_wait_patched", False):
        return
    _orig = _bu.compile_bir_kernel

    def _patched(bir_json, tmpdir, neff_name="file.neff"):
        return _orig(_legalize_single_wait(bir_json), tmpdir, neff_name)

    _b2j.compile_bir_kernel = _patched
    _b2j._single_wait_patched = True


def _bcast_ap(ap2d, count, pos):
    """Insert a [0, count] broadcast dim at free position `pos`."""
    dims = [list(d) for d in ap2d.ap]
    dims.insert(pos, [0, count])
    return bass.AP(tensor=ap2d.tensor, offset=ap2d.offset, ap=dims)


def _ktjump(tile_ap, off, delta, ncols):
    """rhs AP [128][kt: stride delta, 2][1, ncols] rooted at column `off`."""
    sl = tile_ap[:, off : off + 1]
    return bass.AP(
        tensor=sl.tensor,
        offset=sl.offset,
        ap=[list(sl.ap[0]), [delta, 2], [1, ncols]],
    )


def _build(b2_scaled):
    nc = bass.Bass()

    # DMA carries only pdt + x1 rows (XQB is pure replication, built on-chip)
    big = nc.dram_tensor("big", (128, XQB), F8, kind="ExternalInput")
    # wabq: wab [c, a/b, kt, h, m] flat | xq | w2 (bf16 bits) | b1 (f32 bits)
    # -> one wave-1 chunk carries every weight the first groups need.
    NWQ = 2048 + IBLK + 8 + 16
    wabq = nc.dram_tensor("wabq", (128, NWQ), F8, kind="ExternalInput")
    cpack = nc.dram_tensor("cpack", (128, 516), BF16, kind="ExternalInput")
    out = nc.dram_tensor("out", (IBLK, H * HS), F32, kind="ExternalOutput")

    with tile.TileContext(nc) as tc, ExitStack() as ctx:
        const = ctx.enter_context(tc.tile_pool(name="const", bufs=1))
        gpool = ctx.enter_context(tc.tile_pool(name="gpool", bufs=4))
        psg = ctx.enter_context(tc.tile_pool(name="psg", bufs=2, space="PSUM"))
        pssc = ctx.enter_context(tc.tile_pool(name="pssc", bufs=1, space="PSUM"))
        psepi = ctx.enter_context(tc.tile_pool(name="psepi", bufs=1, space="PSUM"))

        # ---------- constants (DMAs spread over the 3 DMA-capable queues so
        # triggers (~0.6us each) and transfers overlap; each queue issues its
        # first-group-critical chunks first) ----------
        wq_sb = const.tile([128, NWQ], F8)
        cp_sb = const.tile([128, 516], BF16)
        big_sb = const.tile([128, NBIG], F8)

        msl = cp_sb[:, 388:389]
        mask_bc = bass.AP(tensor=msl.tensor, offset=msl.offset,
                          ap=[list(msl.ap[0]), [64, 2], [0, H], [1, 64]])
        def w2_ap(h):
            o = 2048 + IBLK + 2 * h
            return wq_sb[:, o : o + 2].bitcast(BF16)

        def b1_ap(h):
            o = 2048 + IBLK + 8 + 4 * h
            return wq_sb[:, o : o + 4].bitcast(F32)

        def wsl(ab, h):
            """lhsT view [c, kt, 128] for the a/b weight pack of head h."""
            sl = wq_sb[:, ab * 1024 + h * 128 : ab * 1024 + h * 128 + 1]
            return bass.AP(tensor=sl.tensor, offset=sl.offset,
                           ap=[list(sl.ap[0]), [512, 2], [1, 128]])

        # Wave 1 (parallel on the 3 trigger queues): every first-group input.
        # Wave 2: the long-pair regions + the epilogue constants.
        nc.gpsimd.dma_start(out=wq_sb, in_=wabq[:])
        nc.sync.dma_start(out=big_sb[:, PDT0 : PDT0 + 4096],
                          in_=big[:, PDT0 : PDT0 + 4096])
        nc.scalar.dma_start(out=big_sb[:, XTS : PDT1 + 4096],
                            in_=big[:, XTS : PDT1 + 4096])
        nc.gpsimd.dma_start(out=big_sb[:, PDT0 + 4096 : XTS],
                            in_=big[:, PDT0 + 4096 : XTS])
        nc.scalar.dma_start(out=big_sb[:, PDT1 + 4096 : XQB],
                            in_=big[:, PDT1 + 4096 : XQB])
        nc.sync.dma_start(out=cp_sb, in_=cpack[:])

        # x1 query-broadcast columns: replicate on-chip (DVE), in the order
        # the pair loop consumes them.
        for p in range(32):
            if p < 16:
                off, ext = XQB + p * 256, 128
            else:
                off, ext = XQB + 4096 + (p - 16) * 512, 256
            src = wq_sb[:, 2048 + 2 * p : 2048 + 2 * p + 1]
            src_bc = bass.AP(tensor=src.tensor, offset=src.offset,
                             ap=[list(src.ap[0]), [1, 2], [0, ext]])
            dst = big_sb[:, off : off + 1]
            dst_ap = bass.AP(tensor=dst.tensor, offset=dst.offset,
                             ap=[list(dst.ap[0]), [ext, 2], [1, ext]])
            nc.vector.tensor_copy(dst_ap, src_bc)

        pt_sb = const.tile([128, 2, H, IBLK], BF16)
        v_sb = const.tile([128, 2, H, HS + 1], BF16)
        recip = const.tile([IBLK, H, 1], F32)
        final_sb = const.tile([IBLK, H * HS], F32)

        # score accumulator [j%128, jb, h, i] - 1 PSUM bank, memset so the
        # never-written j-tiles of short queries exp() to a finite value.
        score_ps = pssc.tile([128, 2, H, IBLK], F32)
        nc.vector.memset(score_ps, 0.0)
        nc.vector.memset(v_sb[:, :, :, HS : HS + 1], 1.0)

        # ---------- v = x @ Wv (+ ones column for Z) ----------
        for jc in range(2):
            v_ps = psepi.tile([128, H, HS], F32, tag="vps", name=f"v{jc}")
            nc.tensor.matmul(v_ps, lhsT=cp_sb[:, 132 + jc * 128 : 260 + jc * 128],
                             rhs=cp_sb[:, 4:132], start=True, stop=True)
            nc.vector.tensor_copy(v_sb[:, jc, :, 0:HS], v_ps)

        # ---------- main loop ----------
        # groups: 4 short super-groups (8 queries, ext 128) + 8 long (4 q, 256)
        groups = [("s", sg) for sg in range(4)] + [("l", lg) for lg in range(8)]
        pending = []

        def emit_scores(g_t, kind, gi, h):
            # g_t rows: short = 8 queries x (jb=0); long = 4 queries x 2 jb
            nq = 8 if kind == "s" else 4
            njb = 1 if kind == "s" else 2
            q0 = 8 * gi if kind == "s" else 32 + 4 * gi
            for qi in range(nq):
                q = q0 + qi
                for jb in range(njb):
                    nc.tensor.matmul(
                        score_ps[:, jb, h, q : q + 1],
                        lhsT=g_t[:, qi * njb + jb, :],
                        rhs=w2_ap(h),
                        start=True, stop=True,
                    )

        for kind, gi in groups:
            if kind == "s":
                npair, ext = 4, 128
                pairs = [4 * gi + pp for pp in range(4)]
            else:
                npair, ext = 2, 256
                pairs = [16 + 2 * gi + pp for pp in range(2)]
            for h in range(H):
                ps = psg.tile([128, 8, 128], F32, tag="pre", name=f"ps{kind}{gi}_{h}")
                for pp, p in enumerate(pairs):
                    if kind == "s":
                        o0 = PDT0 + p * 256
                        da = XTS - o0
                        out_sl = ps[:, 2 * pp : 2 * pp + 2, :]
                    else:
                        o0 = PDT0 + 4096 + (p - 16) * 512
                        da = XTL - o0
                        out_sl = ps[:, 4 * pp : 4 * pp + 4, :]
                    nc.tensor.matmul(out_sl, lhsT=wsl(0, h),
                                     rhs=_ktjump(big_sb, o0, da, 2 * ext),
                                     start=True, stop=False, perf_mode=DR)
                    nc.tensor.matmul(out_sl, lhsT=wsl(1, h),
                                     rhs=_ktjump(big_sb, o0 + PDT1, DB, 2 * ext),
                                     start=False, stop=True, perf_mode=DR)
                g_t = gpool.tile([128, 8, 128], BF16, tag="g", name=f"g{kind}{gi}_{h}")
                nc.scalar.activation(out=g_t, in_=ps, func=GELU,
                                     bias=b1_ap(h), scale=1.0)
                pending.append((g_t, kind, gi, h))
                if len(pending) > 2:
                    emit_scores(*pending.pop(0))
        while pending:
            emit_scores(*pending.pop(0))

        # ---------- epilogue ----------
        for h in range(H):
            nc.scalar.activation(out=pt_sb[:, :, h, :], in_=score_ps[:, :, h, :],
                                 func=EXP, bias=float(b2_scaled[h]), scale=SCALE)
        nc.vector.tensor_mul(pt_sb, pt_sb, mask_bc)

        av = psepi.tile([IBLK, H, HS + 1], F32, tag="av", name="av")
        for h in range(H):
            for jc in range(2):
                nc.tensor.matmul(av[:, h, :], lhsT=pt_sb[:, jc, h, :],
                                 rhs=v_sb[:, jc, h, :],
                                 start=(jc == 0), stop=(jc == 1))
        nc.vector.reciprocal(out=recip, in_=av[:, :, HS : HS + 1])
        for h in range(H):
            nc.vector.tensor_scalar_mul(
                final_sb[:, h * HS : (h + 1) * HS], av[:, h, 0:HS], recip[:, h, :]
            )
        nc.sync.dma_start(out=out[:], in_=final_sb)

    return nc


def _prep_core(x1t_b, pd_b, k):
    """Build the per-core big-tile columns (fp8) and mask for residue k."""
    f8 = ml_dtypes.float8_e4m3fn
    qsel = 4 * np.arange(IBLK) + k
    arr = pd_b[qsel].transpose(2, 0, 1)            # (256 c2, 64 q, 256 j)
    regions = []
    for kt in range(2):
        ktarr = arr[kt * 128 : (kt + 1) * 128]
        short = ktarr[:, 0:32, 0:128].reshape(128, 4096)
        longr = ktarr[:, 32:64, :].reshape(128, 8192)
        regions.append(np.concatenate([short, longr], axis=1))
    x1qs = np.ascontiguousarray(x1t_b[:, qsel]).astype(f8)   # (128, 64)
    xts = np.tile(x1t_b[:, 0:128], (1, 2))
    xtl = np.tile(x1t_b, (1, 2))
    bigc = np.concatenate(
        [regions[0], xts, xtl, regions[1]], axis=1).astype(f8)

    jidx = np.arange(128)[:, None, None] + np.array([0, 128])[None, :, None]
    mask = (jidx <= (4 * np.arange(IBLK) + k)[None, None, :]).astype(
        ml_dtypes.bfloat16)
    return bigc, x1qs, mask


def kernel(**inputs):
    x = np.asarray(inputs["x"], np.float32)
    st = np.asarray(inputs["st_pos_emb"], np.float32)
    pd = np.asarray(inputs["pos_dist_emb"], np.float32)
    W1 = np.asarray(inputs["W1"], np.float32)
    b1 = np.asarray(inputs["b1"], np.float32)
    W2 = np.asarray(inputs["W2"], np.float32)
    b2 = np.asarray(inputs["b2"], np.float32)
    Wv = np.asarray(inputs["Wv"], np.float32)
    bv = np.asarray(inputs["bv"], np.float32)

    bf = ml_dtypes.bfloat16
    f8 = ml_dtypes.float8_e4m3fn
    x1 = x + st[None]                                    # (B, T, C)
    x1t_b = np.ascontiguousarray(x1.transpose(0, 2, 1))  # (B, C, T)

    W1k = W1[:, :C, :]                                   # (H, C, C)
    W1q = W1[:, C : 2 * C, :]
    W1p = W1[:, 2 * C :, :]                              # (H, 2C, C)
    wa_a = np.stack([W1p[:, 0:128, :], W1k], axis=0)     # (kt, H, c, m)
    wb_a = np.stack([W1p[:, 128:256, :], W1q], axis=0)
    wab_a = np.ascontiguousarray(
        np.stack([wa_a, wb_a], axis=0).transpose(3, 0, 1, 2, 4)
    ).astype(f8)                                         # (c, a/b, kt, h, m)
    w2_a = np.ascontiguousarray(W2.T).astype(bf)         # (C, H)
    b1_a = np.ascontiguousarray(b1.T)                    # (C, H)
    wv_a = Wv.transpose(1, 0, 2).reshape(C, H * HS).astype(bf)

    key = tuple(float(v) * SCALE for v in b2)
    if key not in _build_cache:
        _build_cache[key] = _build(key)
    nc = _build_cache[key]

    in_maps = []
    for core in range(NCORES):
        b, k = divmod(core, 4)
        bigc, x1qs, mask = _prep_core(x1t_b[b], pd[b], k)
        cpack = np.concatenate(
            [w2_a, wv_a, x[b].T.astype(bf), mask.reshape(128, 128)], axis=1)
        wabq = np.concatenate(
            [wab_a.reshape(128, 2048), x1qs, w2_a.view(f8), b1_a.view(f8)],
            axis=1)
        in_maps.append({
            "big": bigc, "wabq": np.ascontiguousarray(wabq),
            "cpack": np.ascontiguousarray(cpack),
        })

    _install_wait_legalizer()
    res = run_bass_kernel_spmd(nc, in_maps, core_ids=list(range(NCORES)))
    outp = np.zeros((B, T, H * HS), np.float32)
    for core in range(NCORES):
        b, k = divmod(core, 4)
        outp[b, 4 * np.arange(IBLK) + k] = res.results[core]["out"]
    outp += bv.reshape(-1)[None, None, :]
    return outp
